# revision 20
# baseline (speedup 1.0000x reference)
"""Trainium2 Bass kernel for CapsNet conv + dynamic-routing block.

Math note: in the reference, `pred` has a singleton MI axis, so the
softmax-weighted sum over MI is `pred` itself for any routing logits
(softmax rows sum to 1), and the `b` updates never change `c`.  The whole
module therefore reduces exactly to

    out = squash(conv2d_3x3(x2, conv_w) + conv_b)   # squash over DO

with x2 = x reshaped [B, MI*DI, H, W] and output [B, MO, H, W, DO].

Strategy: data-parallel over batch (1 image per NeuronCore, 8 cores).
Conv runs in bf16 (tolerance 2e-2; bf16 conv lands ~5e-3): halves DMA
traffic, transposes at 1.0 cyc/row.  Per core:
  1. A dozen tiny warm-up matmuls on a zeroed tile start the PE p-state
     ramp while the first DMAs are still in flight.
  2. x[b] arrives as 8 row-group DMAs into a zero-padded [128, 66, 66]
     bf16 image; weights as 3 kh-slice DMAs so tap-0 weights land first.
  3. 3x3 conv, one 512-pixel chunk at a time: 9 accumulating bf16
     matmuls (lhsT = W[s][ci,co] stationary, rhs = shifted window of
     x_pad), f32 PSUM.  Chunk c's post-processing is emitted after chunk
     c+1's matmuls so the PE transposes never stall the conv stream.
  4. post: ACT bias-add (casts bf16) -> PE transpose to [pix, co] ->
     Pool square -> DVE sum over DO -> ACT sqrt -> DVE factor chain ->
     DVE scale -> bf16 store per chunk (host casts back to f32).  The
     factor chain for chunk c is emitted after chunk c+1's square so the
     ACT sqrt round-trip never idles the DVE.
"""

from contextlib import ExitStack

import numpy as np
import ml_dtypes

import concourse.bass as bass
import concourse.mybir as mybir
import concourse.tile as tile
from concourse import bacc
from concourse.bass_utils import run_bass_kernel_spmd
from concourse.masks import make_identity

B, MI, H, W, DI = 8, 8, 64, 64, 16
MO, DO = 8, 16
CI = MI * DI  # 128
CO = MO * DO  # 128
P = 128
HP, WP = H + 2, W + 2  # 66 (zero pad = 1)
NCHUNK = 8  # 512-pixel chunks per 64x64 image
EPS = 1e-7

F32 = mybir.dt.float32
BF16 = mybir.dt.bfloat16


def _body(tc, x_in, w_in, b_in, out_d, reps=1):
    nc = tc.nc
    with ExitStack() as ctx:
        consts = ctx.enter_context(tc.tile_pool(name="consts", bufs=1))
        cpsum = ctx.enter_context(tc.tile_pool(name="cpsum", bufs=3, space="PSUM"))
        opsum = ctx.enter_context(tc.tile_pool(name="opsum", bufs=4, space="PSUM"))
        wpsum = ctx.enter_context(tc.tile_pool(name="wpsum", bufs=1, space="PSUM"))
        work = ctx.enter_context(tc.tile_pool(name="work", bufs=3))

        # p-state warm-up: tiny matmuls on a zeroed tile anchor the PE's
        # DVFS ramp clock while the x/w DMAs are still in flight.
        wrm = consts.tile([P, 64], BF16)
        nc.vector.memset(wrm[:], 0.0)
        wps = wpsum.tile([8, 64], F32)
        for _ in range(12):
            nc.tensor.matmul(wps[:], wrm[:, :8], wrm[:], start=True, stop=True)

        # weights: [ci, s, co] bf16, 3 kh-slices so s=0..2 arrive first
        # (ACT HWDGE ring, parallel with x row-groups on the SP ring).
        w_sb = consts.tile([P, 9, CO], BF16)
        for k in range(3):
            nc.scalar.dma_start(w_sb[:, 3 * k : 3 * k + 3, :], w_in[:, 3 * k : 3 * k + 3, :])

        bias_sb = consts.tile([P, 1], F32)
        nc.scalar.dma_start(bias_sb[:], b_in)

        # padded input image [ci, hp, wp] bf16; zero the 1-wide border.
        xpad = consts.tile([P, HP, WP], BF16)
        nc.vector.memset(xpad[:, 0, :], 0.0)
        nc.vector.memset(xpad[:, HP - 1, :], 0.0)
        nc.vector.memset(xpad[:, :, 0], 0.0)
        nc.vector.memset(xpad[:, :, WP - 1], 0.0)

        identity_f32 = consts.tile([P, P], F32)
        make_identity(nc, identity_f32[:])
        identity_bf = consts.tile([P, P], BF16)
        nc.scalar.copy(identity_bf[:], identity_f32[:])

        eps_sb = consts.tile([P, 1], F32)
        nc.vector.memset(eps_sb[:], EPS)

        def load_eighth(g):
            """DMA 8 h-rows of x (contiguous source) into xpad rows 8g+1..8g+9."""
            nc.sync.dma_start(
                xpad[:, 1 + 8 * g : 9 + 8 * g, 1:65],
                x_in[:, 512 * g : 512 * g + 512].rearrange("ci (r w) -> ci r w", w=W),
            )

        out_sb = consts.tile([P, NCHUNK, 4, CO], BF16)

        def conv_chunk(c):
            ps = cpsum.tile([P, 4 * P], F32, tag="ps")
            for s in range(9):
                kh, kw = s // 3, s % 3
                rhs = xpad[:, 8 * c + kh : 8 * c + kh + 8, kw : kw + 64]
                nc.tensor.matmul(
                    ps[:], w_sb[:, s, :], rhs, start=(s == 0), stop=(s == 8)
                )
            return ps

        def bias_chunk(ps, halves=False):
            # PSUM -> SBUF with bias add (ACT, per-partition bias AP),
            # casting to bf16 so the transposes run at 1.0 cyc/row.
            # halves=True emits two half-width adds so the tail transposes
            # can start after only half the copy.
            s_sb = work.tile([P, 4 * P], BF16, tag="s_sb")
            if halves:
                nc.scalar.add(s_sb[:, : 2 * P], ps[:, : 2 * P], bias_sb[:])
                nc.scalar.add(s_sb[:, 2 * P :], ps[:, 2 * P :], bias_sb[:])
            else:
                nc.scalar.add(s_sb[:], ps[:], bias_sb[:])
            return s_sb

        def post_a(c, s_sb, t0=0, t1=4):
            # transpose [t0:t1] 128-px blocks to [pix, co]; square (ACT);
            # sum over DO (DVE).  t0/t1 allow half-chunk posts at the tail.
            nt = t1 - t0
            so_full = opsum.tile([P, 4, P], BF16, tag="so")
            so = so_full[:, :nt, :]
            for t in range(nt):
                nc.tensor.transpose(
                    so[:, t, :], s_sb[:, (t0 + t) * P : (t0 + t + 1) * P],
                    identity_bf[:],
                )
            sq = work.tile([P, nt, P], BF16, tag=f"sq{nt}")
            nc.scalar.square(sq[:], so[:])
            red = work.tile([P, nt * MO], BF16, tag=f"red{nt}")
            # bf16 out keeps the DVE in its 2x mode; ~0.4% on the squash
            # factor is far inside the 2e-2 tolerance.
            with nc.allow_low_precision(reason="squash norm tolerates bf16"):
                nc.vector.tensor_reduce(
                    red[:],
                    sq[:].rearrange("p t (g do) -> p (t g) do", do=DO),
                    axis=mybir.AxisListType.X,
                    op=mybir.AluOpType.add,
                )
            return so, red

        def post_b(c, so, red, t0=0, t1=4, tail=False):
            # factor = red / ((1+red) * sqrt(red+eps)); scale + store.
            # The ACT sqrt sits two pipeline stages behind its reduce, so
            # neither ACT nor DVE ever stalls on the cross-engine hop.
            nt = t1 - t0
            r = work.tile([P, nt * MO], BF16, tag=f"r{nt}")
            nc.scalar.activation(
                r[:], red[:], mybir.ActivationFunctionType.Sqrt, bias=eps_sb[:]
            )
            d = work.tile([P, nt * MO], BF16, tag=f"d{nt}")
            nc.vector.scalar_tensor_tensor(
                d[:], red[:], 1.0, r[:], mybir.AluOpType.add, mybir.AluOpType.mult
            )
            rcp = work.tile([P, nt * MO], BF16, tag=f"rcp{nt}")
            with nc.allow_low_precision(reason="squash factor tolerates bf16"):
                nc.vector.reciprocal(rcp[:], d[:])
            fac = work.tile([P, nt * MO], BF16, tag=f"fac{nt}")
            nc.vector.tensor_mul(fac[:], red[:], rcp[:])
            nc.vector.tensor_mul(
                out_sb[:, c, t0:t1].rearrange("p t (g do) -> p (t g) do", do=DO),
                so.rearrange("p t (g do) -> p (t g) do", do=DO),
                fac[:, :, None].to_broadcast((P, nt * MO, DO)),
            )
            # mid-stream stores ride the software DGE (Pool engine is idle
            # there and a SEQ-blocking wait is harmless); tail stores use
            # the SP HWDGE ring, which has no loads left to block.
            if tail:
                nc.sync.dma_start(out_d[:, c, t0:t1], out_sb[:, c, t0:t1])
            else:
                nc.gpsimd.dma_start(out_d[:, c, t0:t1], out_sb[:, c, t0:t1])

        def one_image():
            # chunk c reads xpad rows 8c..8c+10, i.e. eighths up to c+1;
            # keep the loads two chunks ahead of the matmuls.  Emission
            # per chunk: mm(c), bias(c), post_a(c-1), post_b(c-2) — a
            # depth-2 software pipeline; the last chunk posts in halves.
            for g in range(3):
                load_eighth(g)
            sbufs = {}  # c -> s_sb
            a_state = {}  # c -> (so, red)
            for c in range(NCHUNK - 1):
                if 1 <= c <= 5:
                    load_eighth(c + 2)
                ps = conv_chunk(c)
                # post_a(c-1) BEFORE bias(c) on the ACT queue: T(c-1) runs
                # at mm(c)'s end, so square(c-1) is ready right away and
                # the DVE reduce isn't pushed a chunk late.
                if c >= 1:
                    a_state[c - 1] = post_a(c - 1, sbufs.pop(c - 1))
                sbufs[c] = bias_chunk(ps)
                if c >= 2:
                    post_b(c - 2, *a_state.pop(c - 2))
            # drain: chunk 7 biases and posts in half-chunks so the tail's
            # serial chain is half-width all the way down
            ps7 = conv_chunk(7)
            a_state[6] = post_a(6, sbufs.pop(6))
            s7 = bias_chunk(ps7, halves=True)
            post_b(5, *a_state.pop(5))
            a7a = post_a(7, s7, 0, 2)
            post_b(6, *a_state.pop(6), tail=True)
            a7b = post_a(7, s7, 2, 4)
            post_b(7, *a7a, 0, 2, tail=True)
            post_b(7, *a7b, 2, 4, tail=True)

        if reps == 1:
            one_image()
        else:
            with tc.For_i(0, reps, 1):
                one_image()


_NC_CACHE = {}


def _get_nc(reps=1):
    key = ("nc", reps)
    if key not in _NC_CACHE:
        nc = bacc.Bacc("TRN2", target_bir_lowering=False, debug=False, num_devices=8)
        x_in = nc.dram_tensor("x", [CI, H * W], BF16, kind="ExternalInput").ap()
        w_in = nc.dram_tensor("w", [CI, 9, CO], BF16, kind="ExternalInput").ap()
        b_in = nc.dram_tensor("bias", [CO, 1], F32, kind="ExternalInput").ap()
        out_d = nc.dram_tensor("out", [P, NCHUNK, 4, CO], BF16, kind="ExternalOutput").ap()
        with tile.TileContext(nc) as tc:
            _body(tc, x_in, w_in, b_in, out_d, reps=reps)
        nc.compile()
        _NC_CACHE[key] = nc
    return _NC_CACHE[key]


def run(x, conv_w, conv_b, trace=False, reps=1):
    nc = _get_nc(reps=reps)
    # shard/prep: channel-major x per image, [ci, pix] contiguous, bf16
    xt = np.ascontiguousarray(
        np.asarray(x, dtype=np.float32)
        .transpose(0, 1, 4, 2, 3)
        .reshape(B, CI, H * W)
        .astype(ml_dtypes.bfloat16)
    )
    w9 = np.ascontiguousarray(
        np.asarray(conv_w, dtype=np.float32)
        .reshape(CO, CI, 9)
        .transpose(1, 2, 0)
        .astype(ml_dtypes.bfloat16)
    )
    bias = np.ascontiguousarray(np.asarray(conv_b, dtype=np.float32).reshape(CO, 1))
    in_maps = [{"x": xt[b], "w": w9, "bias": bias} for b in range(B)]
    res = run_bass_kernel_spmd(nc, in_maps, list(range(B)), trace=trace)
    # gather/unshard: out_dev[p, c, t, mo, do] -> out[b, mo, h, w, do]
    # with h = 8c + 2t + p//64, w = p%64
    dev = np.stack(
        [res.results[i]["out"].astype(np.float32) for i in range(B)], axis=0
    )
    dev = dev.reshape(B, 2, W, NCHUNK, 4, MO, DO)  # [b, hl, w, c, t, mo, do]
    out = np.ascontiguousarray(
        dev.transpose(0, 5, 3, 4, 1, 2, 6).reshape(B, MO, H, W, DO)
    )
    return out, res


def kernel(x, conv_w, conv_b, b_logits=None, **_ignored):
    # b_logits provably has no effect on the reference output (see module
    # docstring), so it is accepted and ignored.
    out, _ = run(x, conv_w, conv_b, trace=False)
    return out
